# revision 1
# baseline (speedup 1.0000x reference)
"""8-core Trainium2 Bass kernel for nn_Attention_54778012893378.

Tensor-parallel over heads (2 heads/core). Per core:
  phase 1: q/k/v projections from x^T (fp32r matmuls), RoPE fused with the
           per-query score scaling (section_log_len * seq_scale / sqrt(hd)
           folded into q's cos/sin tables on host), q/k kept transposed
           [hd, seq] so scores come out kpos-major.
  phase 2: scores^T tiles [kpos,128 x q,512] = k^T.T @ q^T (fp32r), softmax
           without max-subtraction: exp on ScalarE -> bf16, mask applied as
           a bf16 multiply by exp(mask)^T (host-precomputed; causal path uses
           4 shared diagonal patterns and skips fully-masked tiles),
           denominator via DVE accumulation + one ones-matmul, PV matmul
           (bf16) gives attn^T [hd, q] directly.
  phase 3: normalize (the ones-matmul uses a [128,128] stationary so the
           denominator lands pre-broadcast in PSUM; fast-approx reciprocal +
           multiply), bf16 payload per-head AllToAlls so core i ends with
           attn^T[:, rows_i] for all global head-columns; head-0's exchange
           and half the output projection overlap head-1's attention.
  phase 4: out_rows_i = attn^T.T @ wo (bf16), row-sharded output; host
           concatenates the 8 row shards. A tiny warm-up collective runs
           during phase 1 to absorb first-collective setup; wo streams on
           the gpsimd queue gated on phase-1 end.
"""

import numpy as np
import ml_dtypes

import concourse.bass as bass
import concourse.bacc as bacc
import concourse.tile as tile
import concourse.mybir as mybir
from concourse.bass_utils import run_bass_kernel_spmd

F32 = mybir.dt.float32
F32R = mybir.dt.float32r
BF16 = mybir.dt.bfloat16
AF = mybir.ActivationFunctionType
bf16 = ml_dtypes.bfloat16

# problem dims (hardcoded per spec)
S, D, H, HD, NC = 2048, 2048, 16, 128, 8
HL = H // NC            # local heads per core
CW = HL * HD            # per-core head-column width
RW = S // NC            # per-core output row width


def _rope_drain(nc, rtmp, ps, out_sl, cs, cs_w, w):
    """Full-width rope from psum [te;to]: oe = te*c - to*s ; oo = te*s + to*c.
    cs = [c;s] packed [128, w]; cs_w = [s;c] (swapped). Mixed psum+sbuf
    operands may differ in base partition; both-sbuf operands may not, so
    the upper halves are staged through base-0 copies."""
    u1 = rtmp.tile([128, w], F32, tag="u1", name="u1")
    u2 = rtmp.tile([128, w], F32, tag="u2", name="u2")
    nc.vector.tensor_mul(u1, ps, cs)         # [te*c ; to*s]
    nc.vector.tensor_mul(u2, ps, cs_w)       # [te*s ; to*c] -- frees psum
    b1 = rtmp.tile([64, w], F32, tag="b1", name="b1")
    b2 = rtmp.tile([64, w], F32, tag="b2", name="b2")
    nc.vector.tensor_copy(b1, u1[64:128, :])
    nc.vector.tensor_copy(b2, u2[64:128, :])
    nc.vector.tensor_sub(out_sl[0:64, :], u1[0:64, :], b1)
    nc.vector.tensor_add(out_sl[64:128, :], u2[0:64, :], b2)


def build_nc(causal, s=S, d=D, qc_w=512):
    assert HL == 2, "cq/sq packing assumes 2 local heads"
    ndt = d // 128          # contraction tiles over model dim
    nkt = s // 128          # kpos tiles
    nqc = s // qc_w         # q chunks
    nsc = s // qc_w         # phase-1 seq chunks (same width)
    nst = qc_w // 128       # kpos tiles per q-chunk band
    rw = s // NC
    sc_w = qc_w
    nj = qc_w // rw         # dest cores covered by one q chunk

    nc = bacc.Bacc("TRN2", target_bir_lowering=False, debug=False, num_devices=NC)

    xt = nc.dram_tensor("xt", [d, s], F32, kind="ExternalInput").ap()
    wq = nc.dram_tensor("wq", [d, CW], F32, kind="ExternalInput").ap()
    wk = nc.dram_tensor("wk", [d, CW], F32, kind="ExternalInput").ap()
    wv = nc.dram_tensor("wv", [d, CW], F32, kind="ExternalInput").ap()
    wo = nc.dram_tensor("wo", [d, d], BF16, kind="ExternalInput").ap()
    cq = nc.dram_tensor("cq", [128, s], F32, kind="ExternalInput").ap()
    sq = nc.dram_tensor("sq", [128, s], F32, kind="ExternalInput").ap()
    cksk = nc.dram_tensor("cksk", [128, s], F32, kind="ExternalInput").ap()
    if causal:
        em = nc.dram_tensor("em", [nst, 128, qc_w], BF16, kind="ExternalInput").ap()
    else:
        em = nc.dram_tensor("em", [s, s], BF16, kind="ExternalInput").ap()
    out = nc.dram_tensor("out", [rw, d], F32, kind="ExternalOutput").ap()

    import contextlib
    from concourse.tile import add_dep_helper

    with tile.TileContext(nc, num_cores=NC) as tc:
        with contextlib.ExitStack() as top:
            qkv = top.enter_context(tc.tile_pool(name="qkv", bufs=1))
            qT_s = qkv.tile([128, HL, s], F32R)
            kT_s = qkv.tile([128, HL, s], F32R)
            v_s = qkv.tile([128, nkt, CW], BF16)
            dram = top.enter_context(tc.tile_pool(name="dram", bufs=1, space="DRAM"))
            a2a_in = [dram.tile([NC, HD, rw], BF16, name=f"a2ain{_h}") for _h in range(HL)]
            a2a_out = [dram.tile([NC, HD, rw], BF16, name=f"a2aout{_h}") for _h in range(HL)]

            # tiny warm-up collective: absorbs the first-collective setup cost
            # on the TOPSP path while phase 1 runs
            warm_i = dram.tile([NC, 1, 64], BF16, name="warm_i")
            warm_o = dram.tile([NC, 1, 64], BF16, name="warm_o")
            wz = qkv.tile([1, NC * 64], BF16)
            nc.vector.memset(wz, 0.0)
            nc.sync.dma_start(warm_i.rearrange("a b c -> b (a c)"), wz)
            nc.gpsimd.collective_compute(
                "AllToAll",
                mybir.AluOpType.bypass,
                replica_groups=[list(range(NC))],
                ins=[warm_i.opt()],
                outs=[warm_o.opt()],
            )

            phase1_last = None  # instruction marking first-seq-chunk completion

            # ---------------- phase 1: projections + rope ----------------
            with contextlib.ExitStack() as p1:
                consts = p1.enter_context(tc.tile_pool(name="p1c", bufs=1))
                cq_s = consts.tile([128, s], F32)     # head0 [c;s] (scaled)
                sq_s = consts.tile([128, s], F32)     # head1 [c;s] (scaled)
                ck_s = consts.tile([128, s], F32)     # k [c;s]
                cqw_s = consts.tile([128, s], F32)    # swapped [s;c]
                sqw_s = consts.tile([128, s], F32)
                ckw_s = consts.tile([128, s], F32)
                wq_sb = consts.tile([128, ndt, CW], F32R)
                wk_sb = consts.tile([128, ndt, CW], F32R)
                wv_sb = consts.tile([128, ndt, CW], F32R)
                wq_r = wq.rearrange("(dt p) c -> dt p c", p=128).bitcast(F32R)
                wk_r = wk.rearrange("(dt p) c -> dt p c", p=128).bitcast(F32R)
                wv_r = wv.rearrange("(dt p) c -> dt p c", p=128).bitcast(F32R)

                xch = p1.enter_context(tc.tile_pool(name="xch", bufs=min(ndt + 4, 2 * ndt)))
                psqk = p1.enter_context(tc.tile_pool(name="psqk", bufs=2 * HL, space="PSUM"))
                psv = p1.enter_context(tc.tile_pool(name="psv", bufs=4, space="PSUM"))
                rtmp = p1.enter_context(tc.tile_pool(name="rtmp", bufs=2))

                xt_r = xt.rearrange("(dt p) z -> dt p z", p=128)
                for sc in range(nsc):
                    scs = slice(sc * sc_w, (sc + 1) * sc_w)
                    xts = []
                    for dt in range(ndt):
                        # interleave weight chunks with x chunks on the first
                        # pass so dt=0 operands land first
                        if sc == 0:
                            nc.sync.dma_start(wq_sb[:, dt, :], wq_r[dt])
                            nc.sync.dma_start(wk_sb[:, dt, :], wk_r[dt])
                        t = xch.tile([128, sc_w], F32R, tag="xch", name=f"xch{sc}_{dt}")
                        nc.sync.dma_start(t, xt_r[dt, :, scs].bitcast(F32R))
                        xts.append(t)
                    if sc == 0:
                        for dt in range(ndt):
                            nc.sync.dma_start(wv_sb[:, dt, :], wv_r[dt])
                        nc.sync.dma_start(cq_s, cq)
                        nc.sync.dma_start(sq_s, sq)
                        nc.sync.dma_start(ck_s, cksk)
                        for src_t, dst_t in ((cq_s, cqw_s), (sq_s, sqw_s), (ck_s, ckw_s)):
                            nc.vector.tensor_copy(dst_t[0:64, :], src_t[64:128, :])
                            nc.vector.tensor_copy(dst_t[64:128, :], src_t[0:64, :])
                    q_ps = [psqk.tile([128, sc_w], F32, tag="psqk", name=f"qps{sc}_{_h}") for _h in range(HL)]
                    k_ps = [psqk.tile([128, sc_w], F32, tag="psqk", name=f"kps{sc}_{_h}") for _h in range(HL)]
                    for dt in range(ndt):
                        fl = dict(start=(dt == 0), stop=(dt == ndt - 1))
                        for h in range(HL):
                            nc.tensor.matmul(
                                q_ps[h],
                                lhsT=wq_sb[:, dt, HD * h : HD * (h + 1)],
                                rhs=xts[dt],
                                **fl,
                            )
                        for h in range(HL):
                            nc.tensor.matmul(
                                k_ps[h],
                                lhsT=wk_sb[:, dt, HD * h : HD * (h + 1)],
                                rhs=xts[dt],
                                **fl,
                            )
                    nvt = sc_w // 128       # seq tiles in this chunk
                    v_ps = [psv.tile([128, CW], F32, tag="psv", name=f"vps{sc}_{_b}") for _b in range(nvt)]
                    for dt in range(ndt):
                        for st in range(nvt):
                            nc.tensor.matmul(
                                v_ps[st],
                                lhsT=xts[dt][:, st * 128 : (st + 1) * 128],
                                rhs=wv_sb[:, dt, :],
                                start=(dt == 0),
                                stop=(dt == ndt - 1),
                            )
                    qcs = (cq_s, sq_s)
                    qcsw = (cqw_s, sqw_s)
                    for h in range(HL):
                        _rope_drain(
                            nc, rtmp, q_ps[h], qT_s[:, h, scs],
                            qcs[h][:, scs], qcsw[h][:, scs], sc_w,
                        )
                        _rope_drain(
                            nc, rtmp, k_ps[h], kT_s[:, h, scs],
                            ck_s[:, scs], ckw_s[:, scs], sc_w,
                        )
                    for st in range(nvt):
                        cp = nc.vector.tensor_copy(
                            v_s[:, sc * nvt + st, :], v_ps[st]
                        )
                        if sc == nsc - 1 and st == nvt - 1:
                            phase1_last = cp.ins

            # ---------------- phase 2: attention ----------------
            with contextlib.ExitStack() as p2:
                c2 = p2.enter_context(tc.tile_pool(name="c2", bufs=1))
                ones_s = c2.tile([128, 128], BF16)
                nc.vector.memset(ones_s, 1.0)
                em_s = None
                if causal:
                    em_s = c2.tile([128, nst, qc_w], BF16)
                    nc.sync.dma_start(em_s, em.rearrange("m p q -> p m q"))

                wop = p2.enter_context(tc.tile_pool(name="wop", bufs=1))
                wo_sb = wop.tile([128, ndt, d], BF16)
                wo_r = wo.rearrange("(kt p) n -> kt p n", p=128)

                pss = p2.enter_context(tc.tile_pool(name="pss", bufs=2, space="PSUM"))
                pso = p2.enter_context(tc.tile_pool(name="pso", bufs=2, space="PSUM"))
                psd = p2.enter_context(tc.tile_pool(name="psd", bufs=2, space="PSUM"))
                psw = p2.enter_context(tc.tile_pool(name="psw", bufs=2, space="PSUM"))
                ep = p2.enter_context(tc.tile_pool(name="ep", bufs=4))
                accp = p2.enter_context(tc.tile_pool(name="accp", bufs=2))
                emp = None
                if not causal:
                    emp = p2.enter_context(tc.tile_pool(name="emp", bufs=4))
                p4 = p2.enter_context(tc.tile_pool(name="p4", bufs=1))
                outp = p2.enter_context(tc.tile_pool(name="outp", bufs=2))

                def attn_head(h, dst):
                    for qc in range(nqc):
                        qsl = slice(qc * qc_w, (qc + 1) * qc_w)
                        n_kt = nst * (qc + 1) if causal else nkt
                        o_ps = pso.tile([128, qc_w], F32, tag="pso", name=f"ops{h}_{qc}")
                        acc = accp.tile([128, qc_w], BF16, tag="acc", name=f"acc{h}_{qc}")
                        for kt in range(n_kt):
                            s_ps = pss.tile([128, qc_w], F32, tag="pss", name=f"sps{h}_{qc}_{kt}")
                            nc.tensor.matmul(
                                s_ps,
                                lhsT=kT_s[:, h, kt * 128 : (kt + 1) * 128],
                                rhs=qT_s[:, h, qsl],
                                start=True,
                                stop=True,
                            )
                            e = ep.tile([128, qc_w], BF16, tag="e", name=f"e{h}_{qc}_{kt}")
                            nc.scalar.activation(e, s_ps, AF.Exp)
                            if causal:
                                m = kt - nst * qc
                                if m >= 0:
                                    nc.vector.tensor_mul(e, e, em_s[:, m, :])
                            else:
                                emt = emp.tile([128, qc_w], BF16, tag="em", name=f"emt{h}_{qc}_{kt}")
                                nc.sync.dma_start(
                                    emt, em[kt * 128 : (kt + 1) * 128, qsl]
                                )
                                nc.vector.tensor_mul(e, e, emt)
                            if kt == 0:
                                nc.vector.tensor_copy(acc, e)
                            else:
                                nc.vector.tensor_add(acc, acc, e)
                            nc.tensor.matmul(
                                o_ps,
                                lhsT=v_s[:, kt, HD * h : HD * (h + 1)],
                                rhs=e,
                                start=(kt == 0),
                                stop=(kt == n_kt - 1),
                            )
                        # denominator, pre-broadcast across partitions by a
                        # [128,128] ones stationary
                        d_ps = psd.tile([128, qc_w], F32, tag="psd", name=f"dps{h}_{qc}")
                        nc.tensor.matmul(d_ps, lhsT=ones_s, rhs=acc, start=True, stop=True)
                        rec = ep.tile([128, qc_w], F32, tag="rb", name=f"rb{h}_{qc}")
                        nc.vector.reciprocal_approx_fast(rec, d_ps)
                        att = ep.tile([128, qc_w], BF16, tag="att", name=f"att{h}_{qc}")
                        nc.vector.tensor_mul(att, o_ps, rec)
                        for j in range(nj):
                            nc.sync.dma_start(
                                dst[qc * nj + j, :, :],
                                att[:, j * rw : (j + 1) * rw],
                            )

                def wo_part(lhs_sb, col, o_acc, final):
                    for mt in range(rw // 128):
                        for nk in range(d // 512):
                            nsl = slice(nk * 512, (nk + 1) * 512)
                            w_ps = psw.tile([128, 512], F32, tag="psw", name=f"wps{col}_{mt}_{nk}")
                            for j in range(NC):
                                nc.tensor.matmul(
                                    w_ps,
                                    lhsT=lhs_sb[:, j, mt * 128 : (mt + 1) * 128],
                                    rhs=wo_sb[:, 2 * j + col, nsl],
                                    start=(j == 0),
                                    stop=(j == NC - 1),
                                )
                            if not final:
                                nc.vector.tensor_copy(o_acc[:, mt, nsl], w_ps)
                            else:
                                o_sb = outp.tile([128, 512], F32, tag="osb", name=f"osb{mt}_{nk}")
                                nc.vector.tensor_add(o_sb, o_acc[:, mt, nsl], w_ps)
                                nc.sync.dma_start(
                                    out[mt * 128 : (mt + 1) * 128, nsl], o_sb
                                )

                # head 0 attention; its all-to-all and the first half of the
                # output projection overlap head-1 attention
                attn_head(0, a2a_in[0])
                for kt in range(ndt):
                    dins = nc.gpsimd.dma_start(wo_sb[:, kt, :], wo_r[kt])
                    if kt == 0 and phase1_last is not None:
                        add_dep_helper(dins.ins, phase1_last,
                                       reason="wo prefetch after phase-1 input stream")
                nc.gpsimd.collective_compute(
                    "AllToAll",
                    mybir.AluOpType.bypass,
                    replica_groups=[list(range(NC))],
                    ins=[a2a_in[0].opt()],
                    outs=[a2a_out[0].opt()],
                )
                lhs1_sb = p4.tile([128, NC, rw], BF16)
                nc.sync.dma_start(lhs1_sb, a2a_out[0].rearrange("j p q -> p j q"))
                o_acc = p4.tile([128, rw // 128, d], F32)

                attn_head(1, a2a_in[1])
                nc.gpsimd.collective_compute(
                    "AllToAll",
                    mybir.AluOpType.bypass,
                    replica_groups=[list(range(NC))],
                    ins=[a2a_in[1].opt()],
                    outs=[a2a_out[1].opt()],
                )
                wo_part(lhs1_sb, 0, o_acc, final=False)
                lhs2_sb = p4.tile([128, NC, rw], BF16)
                nc.sync.dma_start(lhs2_sb, a2a_out[1].rearrange("j p q -> p j q"))
                wo_part(lhs2_sb, 1, o_acc, final=True)

    nc.compile()
    return nc


def host_prep(inputs, s=S, d=D, qc_w=512):
    x = np.ascontiguousarray(np.asarray(inputs["x"], dtype=np.float32)[0])
    wq = np.asarray(inputs["wq"], dtype=np.float32)
    wk = np.asarray(inputs["wk"], dtype=np.float32)
    wv = np.asarray(inputs["wv"], dtype=np.float32)
    wo = np.asarray(inputs["wo"], dtype=np.float32)
    ss = np.asarray(inputs["seq_scale"], dtype=np.float32).reshape(H)
    cos = np.asarray(inputs["freqs_cos"], dtype=np.float32)
    sin = np.asarray(inputs["freqs_sin"], dtype=np.float32)
    mask = np.asarray(inputs["mask"], dtype=np.float32)[0, 0]
    sll = np.asarray(inputs["section_log_len"], dtype=np.float32).reshape(s)

    nst = qc_w // 128
    zero = mask == 0.0
    causal = bool(
        np.array_equal(zero, np.tril(np.ones((s, s), bool)))
        and np.all(mask[~zero] <= -1e8)
    )

    emT = np.exp(np.minimum(mask, 0.0)).T.astype(bf16)  # [kpos, q]
    if causal:
        # the 4 boundary patterns: tile (kt=nst*qc+m, qc) has
        # em[m][dk, dq] = 1 if (128*m + dk) <= dq else 0 -- identical per qc
        em_in = np.ascontiguousarray(emT[0 : nst * 128, 0:qc_w]).reshape(
            nst, 128, qc_w
        )
    else:
        em_in = np.ascontiguousarray(emT)

    perm = np.concatenate([np.arange(0, HD, 2), np.arange(1, HD, 2)])
    xt = np.ascontiguousarray(x.T)
    scale = sll / np.sqrt(HD)
    cksk = np.ascontiguousarray(np.concatenate([cos.T, sin.T], axis=0))
    wo_b = np.ascontiguousarray(wo.astype(bf16))

    in_maps = []
    for i in range(NC):
        wq_s = np.concatenate(
            [
                wq[:, CW * i + HD * h : CW * i + HD * (h + 1)][:, perm]
                for h in range(HL)
            ],
            axis=1,
        )
        wk_s = np.concatenate(
            [
                wk[:, CW * i + HD * h : CW * i + HD * (h + 1)][:, perm]
                for h in range(HL)
            ],
            axis=1,
        )
        wv_s = wv[:, CW * i : CW * (i + 1)]
        # per-head packed [cos; sin] (scaled): cq = head 0, sq = head 1
        cq = np.concatenate(
            [cos.T * (scale * ss[HL * i])[None, :],
             sin.T * (scale * ss[HL * i])[None, :]], axis=0
        )
        sq = np.concatenate(
            [cos.T * (scale * ss[HL * i + 1])[None, :],
             sin.T * (scale * ss[HL * i + 1])[None, :]], axis=0
        )
        in_maps.append(
            {
                "xt": xt,
                "wq": np.ascontiguousarray(wq_s),
                "wk": np.ascontiguousarray(wk_s),
                "wv": np.ascontiguousarray(wv_s),
                "wo": wo_b,
                "cq": np.ascontiguousarray(cq.astype(np.float32)),
                "sq": np.ascontiguousarray(sq.astype(np.float32)),
                "cksk": cksk,
                "em": em_in,
            }
        )
    return in_maps, causal


_NC_CACHE = {}


def _get_nc(causal):
    if causal not in _NC_CACHE:
        _NC_CACHE[causal] = build_nc(causal)
    return _NC_CACHE[causal]


def kernel(**inputs) -> np.ndarray:
    in_maps, causal = host_prep(inputs)
    nc = _get_nc(causal)
    res = run_bass_kernel_spmd(nc, in_maps, core_ids=list(range(NC)))
    rows = [res.results[i]["out"] for i in range(NC)]
    return np.concatenate(rows, axis=0)[None].astype(np.float32)

